# revision 54
# baseline (speedup 1.0000x reference)
"""Causal self-attention (GQA + RoPE) Trainium2 Bass kernel.

Problem: B=2, T=2048, C=2048, H=16 q-heads, HK=4 kv-heads, HD=128.
Sharding: 8 cores = (batch b in {0,1}) x (kv-head group g in {0..3}).
Each core computes its batch's 4 q-heads / 1 kv-head slice end-to-end
(QKV proj -> RoPE -> causal attention -> o-proj partial), returning a
[T, C] partial y; the host sums the 4 group partials per batch.

Schedule (TimelineSim 233.5us vs 239.6us for the previous rev):
 - The serial DMA engine is the startup bottleneck (x 8MB + wq 2MB +
   wk/wv 1MB ~ 31us).  The whole input stream rides ONE queue (ACT) in
   exact FIFO order (cross-queue round-robin order is not
   controllable), and wave-1 carries the maximum eight 512-col
   accumulation chains -- ALL FOUR K chains (j0-j3) + ALL FOUR j=0 Q
   chains (wq re-laid kc-major on the host, streamed in four 512KB
   chunks just ahead of the Q matmuls, which lag the K chains by
   QLAG=4 chunks) -- so the PE consumes ~4096 cycles per chunk
   slot and never starves while the stream runs; also avoids the PE
   clock-ramp resets (0.65/1.2/2.4GHz pstates) that idle gaps cause.
 - HARD HW CONSTRAINT (not modeled by the simulator): only one matmul
   accumulation chain may be open per PSUM bank at a time; interleaved
   chains in one bank silently corrupt all but the last.  Wave banks:
   K j0/j1 -> pk A/B, K j2/j3 -> pq A/B (their ropes need the late
   cos/sin tails; pq is re-needed latest), Q h0-h3 -> py A/B + sc A/B
   (j0 q-ropes are early, freeing the pools the first segments need).
 - Gate: all 8 ropes run first (bias-add on ACT, muls+final add on
   DVE, half-swaps on the free SP queue); V0-7 kc-inner solo chains
   take the banks in rope-release order; V8-15 + their bias-adds run
   as segment-0..2 fillers in the pq pool only (py/pk/sc hold
   long-lived segment tiles -- a fill chain there would deadlock the
   in-order PE behind a bank that frees at segment end).  wo streams
   on the sync queue AFTER the gate-rope swaps (it camped on the DMA
   engine exactly when the swaps needed it otherwise).
 - Softmax denominators: bf16 tree per 4 tk blocks on DVE, pairs of
   group-trees merged with one DVE add, ONE ones-matmul per 8 blocks
   (24 ones-matmuls total) with no extra segment-end latency.
 - Attention segments (h,j) j-outer h-inner; segs >=3 interleave the
   next segment's q-projection as PE filler (exp 612ns/block vs sc+av
   427ns/block); the last segment pre-computes the first o-proj row.
 - o-proj PSUM chunks rotate across all four pools; the final row
   block runs 512/512/512/256/256 chunks so the exposed tail
   (matmul->copy->DMA->drain) is short.
"""
import contextlib
from collections import deque

import numpy as np
import ml_dtypes

import concourse.bass as bass
import concourse.tile as tile
import concourse.mybir as mybir
from concourse.bass_utils import run_bass_kernel_spmd

BF16 = ml_dtypes.bfloat16

B, T, C = 2, 2048, 2048
H, HK, HD = 16, 4, 128
GQ = H // HK            # q heads per core = 4
NCORES = 8
TQC = 512               # tq chunk width
NTQ = T // TQC          # 4
NKC = C // 128          # 16 contraction chunks
NTK = T // 128          # 16 tk blocks
SCALE = 1.0 / float(np.sqrt(HD))

DT = mybir.dt.bfloat16
F32 = mybir.dt.float32


def _split_waits(nc, maxw=1):
    """This walrus build rejects instructions with >1 sync wait; move
    overflow waits onto same-engine nops inserted just before."""
    cnt = 0
    for f in nc.m.functions:
        for bb in f.blocks:
            idx = 0
            while idx < len(bb.instructions):
                inst = bb.instructions[idx]
                si = inst.sync_info
                waits = list(si.on_wait) if si is not None and si.on_wait else []
                if len(waits) > maxw:
                    updates = list(si.on_update) if si.on_update else []
                    keep, rest = waits[:maxw], waits[maxw:]
                    pos = idx
                    while rest:
                        chunk, rest = rest[:maxw], rest[maxw:]
                        cnt += 1
                        nop = mybir.InstNoOp(
                            name=f"waitsplit_{cnt}", engine=inst.engine,
                            ins=[], outs=[])
                        nop.sync_info = mybir.SyncInfo(on_wait=chunk, on_update=[])
                        nc.register_instruction(nop, overwrite=True)
                        bb.instructions.insert(pos, nop)
                        pos += 1
                        idx += 1
                    inst.sync_info = mybir.SyncInfo(on_wait=keep, on_update=updates)
                idx += 1
    return cnt


def build(reps: int = 1):
    nc = bass.Bass(target_bir_lowering=False)
    xTd = nc.dram_tensor("xT", [C, T], DT, kind="ExternalInput")
    cosT = nc.dram_tensor("cosT", [HD, T], DT, kind="ExternalInput")
    sinT = nc.dram_tensor("sinT", [HD, T], DT, kind="ExternalInput")
    wq = nc.dram_tensor("wq", [128, GQ * NKC * HD], DT, kind="ExternalInput")
    wk = nc.dram_tensor("wk", [128, NKC * HD], DT, kind="ExternalInput")
    wv = nc.dram_tensor("wv", [128, NKC * HD], DT, kind="ExternalInput")
    wo = nc.dram_tensor("wo", [128, GQ * C], DT, kind="ExternalInput")
    bqT = nc.dram_tensor("bqT", [HD, GQ], F32, kind="ExternalInput")
    bkT = nc.dram_tensor("bkT", [HD, 1], F32, kind="ExternalInput")
    bvr = nc.dram_tensor("bvr", [1, HD], F32, kind="ExternalInput")
    yp = nc.dram_tensor("yp", [T, C], DT, kind="ExternalOutput")
    rcscr = nc.dram_tensor("rcscr", [GQ * NTQ, TQC], F32)

    with tile.TileContext(nc) as tc, contextlib.ExitStack() as ctx:
        const = ctx.enter_context(tc.tile_pool(name="const", bufs=1))
        xtp = ctx.enter_context(tc.tile_pool(name="xtp", bufs=1))
        resid = ctx.enter_context(tc.tile_pool(name="resid", bufs=1))
        ytnp = ctx.enter_context(tc.tile_pool(name="ytnp", bufs=1))
        stage = ctx.enter_context(tc.tile_pool(name="stage", bufs=2))
        exs = ctx.enter_context(tc.tile_pool(name="exs", bufs=2))
        nrm = ctx.enter_context(tc.tile_pool(name="nrm", bufs=2))
        est = ctx.enter_context(tc.tile_pool(name="est", bufs=6))
        outp = ctx.enter_context(tc.tile_pool(name="outp", bufs=6))
        ps_sc = ctx.enter_context(tc.tile_pool(name="ps_sc", bufs=2, space="PSUM"))
        ps_py = ctx.enter_context(tc.tile_pool(name="ps_py", bufs=2, space="PSUM"))
        ps_pq = ctx.enter_context(tc.tile_pool(name="ps_pq", bufs=2, space="PSUM"))
        ps_pk = ctx.enter_context(tc.tile_pool(name="ps_pk", bufs=2, space="PSUM"))

        # ---- weights / constants (DMA queue order matters; see below) ----
        wk_all = const.tile([128, NKC * HD], DT)
        wv_all = const.tile([128, NKC * HD], DT)
        wq_all = const.tile([128, GQ * NKC * HD], DT)
        wo_all = const.tile([128, GQ * C], DT)
        wk_t = [wk_all[:, kc * HD:(kc + 1) * HD] for kc in range(NKC)]
        wv_t = [wv_all[:, kc * HD:(kc + 1) * HD] for kc in range(NKC)]
        # wq host layout is kc-major: [128p, kc, h, HD] so each kc-chunk
        # (512 cols) can stream just ahead of its wave-1 Q matmuls
        wq_ht = [[wq_all[:, (kc * GQ + h) * HD:(kc * GQ + h + 1) * HD]
                  for kc in range(NKC)] for h in range(GQ)]
        wo_t = [wo_all[:, h * C:(h + 1) * C] for h in range(GQ)]
        bq_sb = const.tile([HD, GQ], F32)
        bk_sb = const.tile([HD, 1], F32)
        bvb_sb = const.tile([128, HD], F32)
        cos_sb = const.tile([HD, T], DT)
        sin_sb = const.tile([HD, T], DT)
        ones_sb = const.tile([128, 1], DT)
        nc.vector.memset(ones_sb, 1.0)
        warm_src = const.tile([128, TQC], DT)
        nc.vector.memset(warm_src, 1.0)

        # per-chunk resident tiles
        xt = [xtp.tile([128, T], DT, tag=f"xt{kc}", name=f"xt{kc}")
              for kc in range(NKC)]
        qTt = [[resid.tile([HD, TQC], DT, tag=f"qT{h}_{j}", name=f"qT{h}_{j}")
                for j in range(NTQ)] for h in range(GQ)]
        kTt = [resid.tile([HD, TQC], DT, tag=f"kT{j}", name=f"kT{j}")
               for j in range(NTQ)]
        vt = [resid.tile([128, HD], DT, tag=f"v{i}", name=f"v{i}")
              for i in range(NTK)]

        def rope_store(psum_src, bias_ap, dst_ap, j0, use_act=False):
            """dst = rope(psum_src + bias).

            sin_sb holds the half-swapped, sign-folded sin (host-prepped:
            rows 0:64 = sin[64:128], rows 64:128 = -sin[0:64]), so
            rot_half reduces to a full-width multiply followed by a
            partition half-swap done with two SBUF->SBUF DMAs.  The
            PSUM->SBUF bias-add runs on ACT (Identity shares the Exp
            table set, so no act-table reloads) unless ACT is the local
            bottleneck (j=3 segments), where it stays on DVE."""
            qs = stage.tile([128, TQC], DT, tag="qs")
            if use_act:
                nc.scalar.activation(
                    out=qs, in_=psum_src,
                    func=mybir.ActivationFunctionType.Identity, bias=bias_ap)
            else:
                nc.vector.tensor_scalar(
                    out=qs, in0=psum_src, scalar1=bias_ap, scalar2=None,
                    op0=mybir.AluOpType.add)
            tmp = stage.tile([128, TQC], DT, tag="tmp")
            nc.vector.tensor_mul(tmp, qs, cos_sb[:, j0:j0 + TQC])
            prod = stage.tile([128, TQC], DT, tag="prod")
            nc.vector.tensor_mul(prod, qs, sin_sb[:, j0:j0 + TQC])
            prodsw = stage.tile([128, TQC], DT, tag="prodsw")
            nc.sync.dma_start(out=prodsw[0:64, :], in_=prod[64:128, :])
            nc.sync.dma_start(out=prodsw[64:128, :], in_=prod[0:64, :])
            nc.vector.tensor_add(dst_ap, tmp, prodsw)

        for rep in range(reps):
            if rep == 0:
                # PE clock warmup: the pstate ramp (0.65/1.2/2.4GHz) needs
                # ~3us of CONTINUOUS busy to reach full speed, and resets on
                # any idle gap.  A dummy chain on memset data keeps the PE
                # busy from ~0.8us while the first DMAs land, so the wave's
                # first real matmuls run at full clock instead of crawling.
                # Occupies sc slot A briefly; released well before its next
                # user (the h2 j0 Q chain) first writes at ~13us.
                warm = ps_sc.tile([1, TQC], F32, tag="sc", name="warm")
                for w in range(10):
                    nc.tensor.matmul(warm, ones_sb, warm_src,
                                     start=(w == 0), stop=(w == 9))
            # ---- input DMAs.  The serial DMA engine round-robins the two
            # queues; the sync queue carries the xt stream in kc order, the
            # scalar queue delivers wq kc-chunks (kc-major host layout) just
            # ahead of each chunk's Q matmuls, so the 8-chain wave below is
            # fed at the stream rate.
            # One logical stream list, strictly alternating the two queues so
            # the serial DMA engine's round-robin reconstructs this order.
            # wq rides in four 512KB kc-major chunks just ahead of the Q
            # matmuls (QLAG=4); gate-time consts (wv, j0 cos/sin) trail the
            # xt chunks; cos/sin tails + wo drain on the scalar queue after.
            stream = []
            QW = 4 * GQ * HD   # wq chunk: 4 kc groups = 512KB

            def xts(kc):
                return (xt[kc], xTd[kc * 128:(kc + 1) * 128, :])
            if rep == 0:
                # xt0 in halves: K j0/j1 of kc=0 start after the first half
                stream += [(wk_all[:, 0:4 * HD], wk[:, 0:4 * HD]),
                           (xt[0][:, 0:2 * TQC], xTd[0:128, 0:2 * TQC]),
                           (xt[0][:, 2 * TQC:], xTd[0:128, 2 * TQC:]),
                           xts(1),
                           (wq_all[:, 0:QW], wq[:, 0:QW]),
                           xts(2),
                           (wk_all[:, 4 * HD:], wk[:, 4 * HD:]),
                           xts(3), xts(4), (bq_sb, bqT[:, :]),
                           xts(5), (bk_sb, bkT[:, :]),
                           xts(6), (bvb_sb,
                                    bass.AP(bvr, 0, [[0, 128], [1, HD]])),
                           xts(7), (wq_all[:, QW:2 * QW], wq[:, QW:2 * QW]),
                           xts(8), xts(9), xts(10),
                           (wq_all[:, 2 * QW:3 * QW], wq[:, 2 * QW:3 * QW]),
                           xts(11), xts(12), xts(13),
                           (wq_all[:, 3 * QW:], wq[:, 3 * QW:]),
                           xts(14), xts(15),
                           (cos_sb[:, 0:TQC], cosT[:, 0:TQC]),
                           (sin_sb[:, 0:TQC], sinT[:, 0:TQC]),
                           (wv_all, wv[:, :]),
                           (cos_sb[:, TQC:], cosT[:, TQC:]),
                           (sin_sb[:, TQC:], sinT[:, TQC:])]
                # wo is NOT in this stream: it would camp on the serial DMA
                # engine right when the gate ropes' half-swap DMAs need it;
                # it is emitted on the sync queue after the gate ropes.
            else:
                stream += [xts(kc) for kc in range(NKC)]
            # single queue (ACT) => strict FIFO order through the serial
            # HWDGE + DMA-engine pipeline; the SP queue stays free for the
            # rope half-swap / output DMAs.
            for dst, src in stream:
                nc.scalar.dma_start(out=dst, in_=src)

            # ---- wave 1 (kc-major, rides the xt stream): ALL FOUR K chains
            # + ALL FOUR j=0 Q chains -- 4096 PE cycles per chunk, matching
            # the ~1.8us DMA slot so the PE never starves while the input
            # stream runs.  Q lags two chunks behind K so each wq kc-chunk
            # (scalar queue) has landed before its Q matmuls issue.
            # HARD CONSTRAINT: one matmul accumulation chain per PSUM bank.
            # Banks: K j0/j1 -> pk A/B, K j2/j3 -> pq A/B (their ropes need
            # the late cos/sin tails; pq is re-needed latest, by the seg-3
            # filler), Q h0/h1 -> py A/B, Q h2/h3 -> sc A/B (j0 ropes are
            # early, freeing the pools the first segments need).
            pk01 = [ps_pk.tile([128, TQC], F32, tag="pk", name=f"pk{j}")
                    for j in range(2)]
            pk23 = [ps_pq.tile([128, TQC], F32, tag="pq", name=f"pk{j+2}")
                    for j in range(2)]
            pkj = pk01 + pk23
            pq0 = [ps_py.tile([128, TQC], F32, tag="py", name=f"pq{h}0")
                   for h in range(2)]
            pq0 += [ps_sc.tile([128, TQC], F32, tag="sc", name=f"pq{h+2}0")
                    for h in range(2)]
            QLAG = 2
            for kc in range(NKC):
                st, sp = kc == 0, kc == NKC - 1
                for j in range(NTQ):
                    nc.tensor.matmul(pkj[j], wk_t[kc],
                                     xt[kc][:, j * TQC:(j + 1) * TQC],
                                     start=st, stop=sp)
                if kc >= QLAG:
                    qc = kc - QLAG
                    for h in range(GQ):
                        nc.tensor.matmul(pq0[h], wq_ht[h][qc],
                                         xt[qc][:, 0:TQC],
                                         start=(qc == 0), stop=False)
            # flush head-major so each Q chain stops as early as possible --
            # the q00 rope (which gates the whole DVE gate sequence) starts
            # ~2.5us sooner than with a kc-major flush
            for h in range(GQ):
                for qc in range(NKC - QLAG, NKC):
                    nc.tensor.matmul(pq0[h], wq_ht[h][qc],
                                     xt[qc][:, 0:TQC],
                                     start=False, stop=(qc == NKC - 1))

            # ---- gate: all 16 V-projection chains run kc-inner (full xt is
            # resident once the stream ends), rotating through banks as the
            # DVE ropes/adds release them.  DVE: j0 q-ropes first (free py/sc
            # for the early segments), kT0 (cos/sin j0 slice), then kT1-3 as
            # their cos/sin tail slices land.
            pv = {}

            def v_chain(i, pool, tag):
                t = pool.tile([128, TQC], F32, tag=tag, name=f"pv{i}")
                pv[i] = t
                for kc in range(NKC):
                    nc.tensor.matmul(
                        t[:, 0:HD], xt[kc][:, i * 128:(i + 1) * 128],
                        wv_t[kc], start=(kc == 0), stop=(kc == NKC - 1))

            def vt_add(i):
                nc.vector.tensor_add(vt[i], pv[i][:, 0:HD], bvb_sb)

            # All eight ropes go first on DVE (bias-add on the idle ACT via
            # use_act) -- each rope releases one wave bank for a V chain;
            # the vt adds follow.  V8-15 run as seg-0..2 fillers, reusing
            # banks freed by the vt adds.
            rope_store(pq0[0], bq_sb[:, 0:1], qTt[0][0], 0, use_act=True)
            v_chain(0, ps_py, "py")      # py-A <- rope q00
            rope_store(pk01[0], bk_sb[:, 0:1], kTt[0], 0, use_act=True)
            v_chain(1, ps_pk, "pk")      # pk-A <- rope kT0
            rope_store(pq0[1], bq_sb[:, 1:2], qTt[1][0], 0, use_act=True)
            v_chain(2, ps_py, "py")      # py-B <- rope q10
            rope_store(pq0[2], bq_sb[:, 2:3], qTt[2][0], 0, use_act=True)
            v_chain(3, ps_sc, "sc")      # sc-A <- rope q20
            rope_store(pq0[3], bq_sb[:, 3:4], qTt[3][0], 0, use_act=True)
            v_chain(4, ps_sc, "sc")      # sc-B <- rope q30
            rope_store(pk01[1], bk_sb[:, 0:1], kTt[1], TQC, use_act=True)
            v_chain(5, ps_pk, "pk")      # pk-B <- rope kT1
            rope_store(pk23[0], bk_sb[:, 0:1], kTt[2], 2 * TQC, use_act=True)
            v_chain(6, ps_pq, "pq")      # pq-A <- rope kT2
            rope_store(pk23[1], bk_sb[:, 0:1], kTt[3], 3 * TQC, use_act=True)
            v_chain(7, ps_pq, "pq")      # pq-B <- rope kT3
            for i in range(8):
                vt_add(i)
            if rep == 0:
                # wo in 8 half-head pieces on the sync queue, behind the
                # gate swaps but ahead of the (slack-tolerant) segment swaps
                for hp in range(2 * GQ):
                    c0 = hp * (C // 2)
                    nc.sync.dma_start(out=wo_all[:, c0:c0 + C // 2],
                                      in_=wo[:, c0:c0 + C // 2])
            # fills rotate through the pq pool only: the active segment holds
            # long-lived tiles in py (accumulator), pk (sums) and sc
            # (pipeline), so a fill chain there would stall the in-order PE
            # behind a bank that frees only at segment end.
            seg_fill = {0: deque(), 1: deque(), 2: deque()}
            for i in range(8, NTK):
                sidx = min((i - 8) // 3, 2)

                def mk_chain(i=i):
                    v_chain(i, ps_pq, "pq")

                def mk_add(i=i):
                    vt_add(i)
                seg_fill[sidx].append(mk_chain)
                seg_fill[sidx].append(mk_add)

            # ---- attention segments ----
            # seg order: j outer, h inner.  seg s handles (h,j); during its
            # blocks the PE is fed fillers: next segment's q-proj, plus the
            # deferred V12-15 / K j2 / K j3 chains.
            segs = [(h, j) for j in range(NTQ) for h in range(GQ)]
            sums_j = {}
            pq_next = {}
            ot00 = {}

            ytn_all = {j: [None] * GQ for j in range(NTQ)}
            for s, (h, j) in enumerate(segs):
                j0 = j * TQC
                nblk = 4 * j + 4
                # fillers: segs 0-2 run the V8-15 projection chains; later
                # segments interleave the next segment's q-projection; the
                # last segment pre-computes the first o-proj row block
                fill = deque()
                if s < 3:
                    fill.extend(seg_fill[s])
                elif s + 1 < len(segs):
                    nh, nj = segs[s + 1]
                    pqn = ps_pq.tile([128, TQC], F32, tag="pq",
                                     name=f"pq{nh}{nj}")
                    pq_next[s + 1] = pqn
                    for kc in range(NKC):
                        fill.append((lambda kc=kc, pqn=pqn, nh=nh, nj=nj:
                                     nc.tensor.matmul(
                                         pqn, wq_ht[nh][kc],
                                         xt[kc][:, nj * TQC:(nj + 1) * TQC],
                                         start=(kc == 0), stop=(kc == NKC - 1))))
                else:
                    ot00[0] = outp.tile([128, C], DT, name="ot00", bufs=1)
                    for cc in range(4):
                        po = ps_pq.tile([128, TQC], F32, tag="pq",
                                        name="po_pre")
                        for hh in range(GQ):
                            def mk(po=po, hh=hh, c0=cc * TQC):
                                nc.tensor.matmul(
                                    po, ytn_all[0][hh][:, 0:128],
                                    wo_t[hh][:, c0:c0 + TQC],
                                    start=(hh == 0), stop=(hh == GQ - 1))
                                if hh == GQ - 1:
                                    nc.vector.tensor_copy(
                                        out=ot00[0][:, c0:c0 + TQC], in_=po)
                            fill.append(mk)

                def after_fill():
                    while fill:
                        fill.popleft()()

                sums_hj = ps_pk.tile([1, TQC], F32, tag="pk",
                                     name=f"sums{h}{j}")
                sums_j[(h, j)] = sums_hj
                py = ps_py.tile([HD, TQC], F32, tag="py", name=f"py{h}{j}")

                # blocks with a 2-deep sc pipeline; group-of-4 tree sums,
                # drained in PAIRS (one DVE merge add) so each ones-matmul
                # covers 8 blocks without extra segment-end latency
                GSZ = 4
                ngrp = (nblk + GSZ - 1) // GSZ
                nmm = (ngrp + 1) // 2
                mm_i = [0]
                ex_t = [None] * nblk
                sc_t = [None] * nblk
                pend_sum = deque()   # (grp, exsum_tile) ready for ones-matmul

                def emit_sc(i):
                    srel = i - 4 * j
                    c0 = 128 * srel if srel > 0 else 0
                    sct = ps_sc.tile([128, TQC], F32, tag="sc")
                    sc_t[i] = (sct, c0)
                    jk, ik = divmod(i, 4)
                    nc.tensor.matmul(
                        sct[:, c0:TQC], kTt[jk][:, ik * 128:(ik + 1) * 128],
                        qTt[h][j][:, c0:TQC], start=True, stop=True)
                    ex = est.tile([128, TQC], DT)
                    ex_t[i] = ex
                    nc.scalar.activation(
                        out=ex[:, c0:TQC], in_=sct[:, c0:TQC],
                        func=mybir.ActivationFunctionType.Exp, scale=SCALE)
                    if srel >= 0:
                        if srel > 0:
                            nc.gpsimd.memset(ex[:, 0:c0], 0.0)
                        # zero below-diagonal of the 128-wide diag sub-block
                        nc.gpsimd.affine_select(
                            out=ex[:, c0:c0 + 128], in_=ex[:, c0:c0 + 128],
                            compare_op=mybir.AluOpType.is_ge, fill=0.0,
                            base=0, pattern=[[1, 128]], channel_multiplier=-1)

                def emit_group_sum(g):
                    lo = g * GSZ
                    hi = min(lo + GSZ, nblk)
                    tiles = [ex_t[i] for i in range(lo, hi)]
                    # in-place bf16 accumulate on DVE with two scratch tiles
                    acc = exs.tile([128, TQC], DT, tag="t01")
                    nc.vector.tensor_add(acc, tiles[0], tiles[1])
                    for a in range(2, len(tiles) - 1, 2):
                        t23 = exs.tile([128, TQC], DT, tag="t23")
                        nc.vector.tensor_add(t23, tiles[a], tiles[a + 1])
                        nc.vector.tensor_add(acc, acc, t23)
                    if len(tiles) % 2:
                        nc.vector.tensor_add(acc, acc, tiles[-1])
                    pend_sum.append((g, acc))

                def emit_pend_sums(merge=True):
                    while pend_sum:
                        g, acc = pend_sum.popleft()
                        if pend_sum and merge:
                            _, acc2 = pend_sum.popleft()
                            nc.vector.tensor_add(acc, acc, acc2)
                        nc.tensor.matmul(
                            sums_hj, ones_sb, acc,
                            start=(mm_i[0] == 0), stop=(mm_i[0] == nmm - 1))
                        mm_i[0] += 1

                emit_sc(0)
                if nblk > 1:
                    emit_sc(1)
                for i in range(nblk):
                    if fill:
                        fill.popleft()()
                    if i + 2 < nblk:
                        emit_sc(i + 2)
                    sct, c0 = sc_t[i]
                    srel = i - 4 * j
                    if srel >= 0 and c0 + 128 < TQC:
                        # diag block: above-diagonal columns need only the
                        # exp (not the Pool select round-trip) -- emit them
                        # first so this av never waits the select.  Still
                        # ONE accumulation chain; disjoint column ranges per
                        # matmul are safe (baseline-proven), unlike
                        # interleaved independent chains.
                        nc.tensor.matmul(
                            py[:, c0 + 128:TQC], vt[i],
                            ex_t[i][:, c0 + 128:TQC],
                            start=(i == 0), stop=False)
                        nc.tensor.matmul(
                            py[:, c0:c0 + 128], vt[i], ex_t[i][:, c0:c0 + 128],
                            start=False, stop=(i == nblk - 1))
                    else:
                        nc.tensor.matmul(
                            py[:, c0:TQC], vt[i], ex_t[i][:, c0:TQC],
                            start=(i == 0), stop=(i == nblk - 1))
                    if (i + 1) % GSZ == 0:
                        emit_group_sum(i // GSZ)
                    if i % 8 == 1 and i > 8:
                        emit_pend_sums()
                after_fill()
                emit_pend_sums()

                # DVE tail: next-q rope first (gates next segment), then
                # normalization of this segment.
                if s + 1 in pq_next:
                    nh, nj = segs[s + 1]
                    # j=3 segments are ACT-limited (16 exps); keep their
                    # rope's PSUM->SBUF step on DVE instead
                    rope_store(pq_next[s + 1], bq_sb[:, nh:nh + 1],
                               qTt[nh][nj], nj * TQC, use_act=(nj != 3))

                # normalize: yT[d, tq] / sum[tq] via DRAM-broadcast of 1/sum
                rc = nrm.tile([1, TQC], F32, tag="rc")
                nc.vector.reciprocal(out=rc, in_=sums_hj[0:1, :])
                rcb = nrm.tile([HD, TQC], F32, tag="rcb")
                idx = h * NTQ + j
                nc.scalar.dma_start(out=rcscr[idx:idx + 1, :], in_=rc)
                nc.scalar.dma_start(
                    out=rcb, in_=bass.AP(rcscr, idx * TQC, [[0, HD], [1, TQC]]))
                yt = ytnp.tile([HD, TQC], DT, tag=f"yt{h}_{j}",
                               name=f"yt{h}_{j}")
                nc.vector.tensor_mul(yt, py, rcb)
                ytn_all[j][h] = yt

            # ---- o-proj: y[tq, :] = sum_h yT_h.T @ Wo_h.  po chunks rotate
            # across all 4 PSUM pools (8 banks) so the ~1.1us PSUM->SBUF
            # copy roundtrip never blocks the 852ns accumulation chains.
            po_pools = [(ps_pq, "pq"), (ps_sc, "sc"), (ps_py, "py"),
                        (ps_pk, "pk")]
            po_i = 0
            nc.sync.dma_start(out=yp[0:128, :], in_=ot00[0])
            for j in range(NTQ):
                ytn = ytn_all[j]
                for t in range(4):  # four 128-row q tiles in this chunk
                    if j == 0 and t == 0:
                        continue  # pre-computed during the last segment
                    trow = j * TQC + t * 128
                    last = (j == NTQ - 1 and t == 3)
                    ot = outp.tile([128, C], DT)
                    # last row: final 512 splits into 256+256 so the very
                    # last matmul->copy->DMA chain is short
                    widths = ([TQC] * 3 + [TQC // 2] * 2) if last \
                        else [TQC] * 4
                    c0 = 0
                    for cc, cw in enumerate(widths):
                        pool, ptag = po_pools[po_i % 4]
                        po_i += 1
                        po = pool.tile([128, cw], F32, tag=ptag, name="po")
                        for hh in range(GQ):
                            nc.tensor.matmul(
                                po, ytn[hh][:, t * 128:(t + 1) * 128],
                                wo_t[hh][:, c0:c0 + cw],
                                start=(hh == 0), stop=(hh == GQ - 1))
                        if (t + cc) % 2 == 0:
                            nc.scalar.copy(out=ot[:, c0:c0 + cw], in_=po)
                        else:
                            nc.vector.tensor_copy(out=ot[:, c0:c0 + cw], in_=po)
                        if last:
                            # fine-grained tail: expose only a short DMA
                            oeng = nc.sync if cc % 2 == 0 else nc.scalar
                            oeng.dma_start(
                                out=yp[trow:trow + 128, c0:c0 + cw],
                                in_=ot[:, c0:c0 + cw])
                        c0 += cw
                    if not last:
                        oeng = nc.sync if t % 2 == 0 else nc.scalar
                        oeng.dma_start(out=yp[trow:trow + 128, :], in_=ot)
    _split_waits(nc, maxw=1)
    return nc


def _in_maps(x, cos, sin, Wq, bq, Wk, bk, Wv, bv, Wo):
    maps = []
    for c in range(NCORES):
        b, g = divmod(c, HK)
        qsl = slice(g * GQ * HD, (g + 1) * GQ * HD)
        ksl = slice(g * HD, (g + 1) * HD)
        maps.append({
            "xT": np.ascontiguousarray(x[b].T.astype(BF16)),
            "cosT": np.ascontiguousarray(cos[b].T.astype(BF16)),
            "sinT": np.ascontiguousarray(np.concatenate(
                [sin[b].T[64:128], -sin[b].T[0:64]], axis=0).astype(BF16)),
            # weights pre-arranged to the exact SBUF layout so every DMA is
            # a contiguous per-partition transfer:
            #   wq: [128p, h, kc, HD] head-major; wk/wv: [128p, kc, HD];
            #   wo: [128p(=HD), h, C]
            "wq": np.ascontiguousarray(
                Wq[:, qsl].reshape(NKC, 128, GQ, HD).transpose(1, 0, 2, 3)
                .reshape(128, GQ * NKC * HD).astype(BF16)),
            "wk": np.ascontiguousarray(
                Wk[:, ksl].reshape(NKC, 128, HD).transpose(1, 0, 2)
                .reshape(128, NKC * HD).astype(BF16)),
            "wv": np.ascontiguousarray(
                Wv[:, ksl].reshape(NKC, 128, HD).transpose(1, 0, 2)
                .reshape(128, NKC * HD).astype(BF16)),
            "wo": np.ascontiguousarray(
                Wo[qsl, :].reshape(GQ, 128, C).transpose(1, 0, 2)
                .reshape(128, GQ * C).astype(BF16)),
            "bqT": np.ascontiguousarray(
                bq[qsl].reshape(GQ, HD).T.astype(np.float32)),
            "bkT": np.ascontiguousarray(
                bk[ksl].reshape(HD, 1).astype(np.float32)),
            "bvr": np.ascontiguousarray(
                bv[ksl].reshape(1, HD).astype(np.float32)),
        })
    return maps


_nc_cache = {}


def kernel(x, cos, sin, Wq, bq, Wk, bk, Wv, bv, Wo):
    x, cos, sin = np.asarray(x), np.asarray(cos), np.asarray(sin)
    Wq, bq = np.asarray(Wq), np.asarray(bq)
    Wk, bk = np.asarray(Wk), np.asarray(bk)
    Wv, bv = np.asarray(Wv), np.asarray(bv)
    Wo = np.asarray(Wo)
    if "nc" not in _nc_cache:
        _nc_cache["nc"] = build(reps=1)
    nc = _nc_cache["nc"]
    maps = _in_maps(x, cos, sin, Wq, bq, Wk, bk, Wv, bv, Wo)
    res = run_bass_kernel_spmd(nc, maps, core_ids=list(range(NCORES)))
    out = np.zeros((B, T, C), dtype=np.float32)
    for c in range(NCORES):
        b = c // HK
        out[b] += res.results[c]["yp"].astype(np.float32)
    return out

